# revision 28
# baseline (speedup 1.0000x reference)
"""Multi-head attention (B=4, S=2048, D=1024, H=16, causal) on 8 TRN2 cores.

Sharding: core c -> (batch b = c//2, head-group g = c%2 of 8 heads).
Each core computes projections for its 8 heads (column-split Wq/Wk/Wv),
flash-style causal attention, and a partial output projection (row-split Wo).
Host unshard sums the two partials per batch and adds bo.

v2 engine plan (PE and ScalarE are the two near-critical engines):
- qc-outer rounds: each round projects one 512-query chunk of q/k/v, runs
  attention for that chunk, then its output projection -- so ScalarE's exp
  stream starts ~20us in and overlaps all later PE work.
- q/k/v slabs + QKV weights stream in as bf16 (halves DMA, same PE rate).
- kwT persists transposed [dout, seq]; qwT/atn chunks rotate. kwT/qwT and
  atn/Wo are bf16 so the scores and out-proj matmuls get Fast Weight Load
  (fp32-family stationary operands disable FWL and double LDWEIGHTS time).
- vw natural [seq, dout] f32r with a ones column per head: the M=65 PV
  matmul emits softmax denominators for free.
- Scores for a head PAIR land in one [128,1024] PSUM tile (2 banks) so ONE
  exp activation covers both heads per j-block (amortizes ACT overhead).
- Causal wedge blocks clamp the query offset so matmul N>=256 (fp32r runs
  4 cyc/row below N=256); the fully-masked overcompute strip is zeroed by
  a DVE memset, the diagonal strip by gpsimd affine_select.
- Normalization: DVE reciprocal of the denominator row, gpsimd
  partition_broadcast, DVE multiply fused into the PSUM->SBUF evacuation.
  ScalarE does nothing but exps.
"""

import os
from contextlib import ExitStack

os.environ.setdefault("JAX_PLATFORMS", "axon")

import numpy as np

import concourse.bass as bass
import concourse.tile as tile
from concourse import bacc, mybir
from concourse.bass_utils import run_bass_kernel_spmd

F32 = mybir.dt.float32
F32R = mybir.dt.float32r
BF16 = mybir.dt.bfloat16
EXP = mybir.ActivationFunctionType.Exp

B, S, D, H = 4, 2048, 1024, 16
HD = D // H          # 64
DL = D // 2          # 512 local douts per core
NT = DL // 128       # 4 dout tiles
NR = S // 128        # 16 row tiles
NQ = S // 512        # 4 query chunks
NDIN = D // 128      # 8 din tiles

LOOKAHEAD = 2        # j-blocks of scores emitted ahead of their PV matmuls


def round_f32r(x):
    """Round fp32 array to the fp32r grid (11 mantissa bits, RNE at bit 12)."""
    u = np.ascontiguousarray(x, np.float32).view(np.uint32)
    r = (u + 0x7FF + ((u >> 12) & 1)) & np.uint32(0xFFFFF000)
    return r.view(np.float32)


def build_nc(reps=1):
    """reps>1 repeats the whole compute in one NEFF (timing calibration)."""
    nc = bacc.Bacc("TRN2", target_bir_lowering=False, debug=False, num_devices=8)

    qT = nc.dram_tensor("qT", [D, S], BF16, kind="ExternalInput").ap()
    kT = nc.dram_tensor("kT", [D, S], BF16, kind="ExternalInput").ap()
    vT = nc.dram_tensor("vT", [D, S], BF16, kind="ExternalInput").ap()
    Wq_s = nc.dram_tensor("Wq_s", [D, DL], BF16, kind="ExternalInput").ap()
    Wk_s = nc.dram_tensor("Wk_s", [D, DL], BF16, kind="ExternalInput").ap()
    Wv_s = nc.dram_tensor("Wv_s", [D, DL], BF16, kind="ExternalInput").ap()
    Wo_s = nc.dram_tensor("Wo_s", [DL, D], BF16, kind="ExternalInput").ap()
    bq_s = nc.dram_tensor("bq_s", [128, NT], F32, kind="ExternalInput").ap()
    bk_s = nc.dram_tensor("bk_s", [128, NT], F32, kind="ExternalInput").ap()
    bv_bc = nc.dram_tensor("bv_bc", [128, DL], F32, kind="ExternalInput").ap()
    out_p = nc.dram_tensor("out_partial", [S, D], BF16, kind="ExternalOutput").ap()

    with tile.TileContext(nc) as tc, ExitStack() as ctx:
        # ---------------- persistent SBUF ----------------
        keep = ctx.enter_context(tc.tile_pool(name="keep", bufs=1))
        kwT = [keep.tile([128, S], BF16, tag=f"kwT{t}", name=f"kwT{t}") for t in range(NT)]
        vw = [keep.tile([128, 8 * 65], F32R, tag=f"vw{r}", name=f"vw{r}") for r in range(NR)]
        bias_q = keep.tile([128, NT], F32, tag="bias_q")  # col t = bq tile t
        bias_k = keep.tile([128, NT], F32, tag="bias_k")
        bv_sb = keep.tile([128, DL], F32, tag="bv_sb")

        wtp = ctx.enter_context(tc.tile_pool(name="wt", bufs=1))
        wop = ctx.enter_context(tc.tile_pool(name="wo", bufs=1))
        slp = ctx.enter_context(tc.tile_pool(name="slab", bufs=1))
        qwp = ctx.enter_context(tc.tile_pool(name="qw", bufs=2))
        anp = ctx.enter_context(tc.tile_pool(name="atn", bufs=2))
        prp = ctx.enter_context(tc.tile_pool(name="probs", bufs=5))
        nrm = ctx.enter_context(tc.tile_pool(name="nrm", bufs=4))
        osp = ctx.enter_context(tc.tile_pool(name="osb", bufs=3))
        pps = ctx.enter_context(tc.tile_pool(name="pps", bufs=2, space="PSUM"))
        scp = ctx.enter_context(tc.tile_pool(name="scps", bufs=2, space="PSUM"))
        atp = ctx.enter_context(tc.tile_pool(name="atps", bufs=1, space="PSUM"))

        # ones column per head (65th PV weight column -> softmax denominators);
        # gpsimd memset on an F32-bitcast view (walrus rejects f32r memsets)
        for r in range(NR):
            ones_ap = vw[r][:].rearrange("p (h e) -> p h e", e=65)[:, :, 64:65]
            nc.gpsimd.memset(ones_ap.bitcast(F32), 1.0)

        # weights/biases ride the (idle) gpsimd DMA queue so the SP queue
        # serves the round-0 activation slabs without queueing behind them
        def load_w(W, pfx):
            w_sb = []
            for dn in range(NDIN):
                w = wtp.tile([128, DL], BF16, tag=f"{pfx}{dn}")
                nc.gpsimd.dma_start(w[:], W[128 * dn:128 * (dn + 1), :])
                w_sb.append(w)
            return w_sb

        wq_sb = load_w(Wq_s, "wq")
        nc.gpsimd.dma_start(bias_q[:], bq_s)
        wk_sb = load_w(Wk_s, "wk")
        nc.gpsimd.dma_start(bias_k[:], bk_s)
        wv_sb = load_w(Wv_s, "wv")
        nc.gpsimd.dma_start(bv_sb[:], bv_bc)
        wo_sb = []
        for t in range(NT):
            w = wop.tile([128, D], BF16, tag=f"wo{t}")
            nc.gpsimd.dma_start(w[:], Wo_s[128 * t:128 * (t + 1), :])
            wo_sb.append(w)

        for rep in range(reps):
          for qc in range(NQ):
            qf = slice(512 * qc, 512 * (qc + 1))
            jmax = 4 * qc + 3

            # ---- project this chunk of q (rotating) and k (persistent) ----
            def load_slab(xT, pfx):
                sl = []
                for dn in range(NDIN):
                    s_ = slp.tile([128, 512], BF16, tag=f"{pfx}{dn}")
                    nc.sync.dma_start(s_[:], xT[128 * dn:128 * (dn + 1), qf])
                    sl.append(s_)
                return sl

            slq = load_slab(qT, "slq")
            slk = load_slab(kT, "slk")
            slv = load_slab(vT, "slv")

            qw_cur = [qwp.tile([128, 512], BF16, tag=f"qw{t}", name=f"qw{t}_{qc}_{rep}")
                      for t in range(NT)]
            for t in range(NT):
                ps = pps.tile([128, 512], F32, tag="pp")
                for dn in range(NDIN):
                    nc.tensor.matmul(
                        ps[:], wq_sb[dn][:, 128 * t:128 * (t + 1)], slq[dn][:],
                        start=(dn == 0), stop=(dn == NDIN - 1))
                nc.vector.tensor_scalar_add(qw_cur[t][:], ps[:], bias_q[:, t:t + 1])
            for t in range(NT):
                ps = pps.tile([128, 512], F32, tag="pp")
                for dn in range(NDIN):
                    nc.tensor.matmul(
                        ps[:], wk_sb[dn][:, 128 * t:128 * (t + 1)], slk[dn][:],
                        start=(dn == 0), stop=(dn == NDIN - 1))
                nc.vector.tensor_scalar_add(
                    kwT[t][:, qf], ps[:], bias_k[:, t:t + 1])

            # ---- project v rows for this chunk (vw natural + ones col) ----
            for r in range(4 * qc, 4 * qc + 4):
                lo = 128 * (r - 4 * qc)
                ps = pps.tile([128, 512], F32, tag="pp")
                for dn in range(NDIN):
                    nc.tensor.matmul(
                        ps[:], slv[dn][:, lo:lo + 128], wv_sb[dn][:],
                        start=(dn == 0), stop=(dn == NDIN - 1))
                dst3 = vw[r][:].rearrange("p (h e) -> p h e", e=65)[:, :, 0:64]
                nc.vector.tensor_add(
                    dst3, ps[:].rearrange("p (h e) -> p h e", e=64),
                    bv_sb[:].rearrange("p (h e) -> p h e", e=64))

            # ---- attention for this chunk ----
            atn_cur = [anp.tile([128, 512], BF16, tag=f"atn{t}", name=f"atn{t}_{qc}_{rep}")
                       for t in range(NT)]
            for p in range(NT):  # head pair p -> local heads (2p, 2p+1)
                atA = atp.tile([65, 512], F32, tag="atA")
                atB = atp.tile([65, 512], F32, tag="atB")

                def scores(j):
                    off_t = max(0, 128 * j - 512 * qc)  # true causal offset
                    off = min(off_t, 256)               # keep matmul N >= 256
                    sc_t = scp.tile([128, 1024], F32, tag="sc")
                    nc.tensor.matmul(
                        sc_t[:, off:512],
                        kwT[p][0:64, 128 * j:128 * (j + 1)],
                        qw_cur[p][0:64, off:512],
                        start=True, stop=True, tile_position=(0, 0))
                    nc.tensor.matmul(
                        sc_t[:, 512 + off:1024],
                        kwT[p][64:128, 128 * j:128 * (j + 1)],
                        qw_cur[p][64:128, off:512],
                        start=True, stop=True, tile_position=(64, 0))
                    pr = prp.tile([128, 1024], F32R, tag="pr")
                    sc3 = sc_t[:].rearrange("p (h n) -> p h n", n=512)
                    pr3 = pr[:].rearrange("p (h n) -> p h n", n=512)
                    nc.scalar.activation(pr3[:, :, off:512], sc3[:, :, off:512],
                                         EXP, scale=1.0 / 8.0)
                    if off_t > off:
                        # fully-masked overcompute strip [off, off_t)
                        pm = pr[:].rearrange("p (h n) -> p h n", n=512)
                        nc.vector.memset(pm[:, :, off:off_t].bitcast(F32), 0.0)
                    if 128 * j >= 512 * qc:
                        # diagonal strip [off_t, off_t+128): keep col >= row
                        for h in range(2):
                            nc.gpsimd.affine_select(
                                out=pr[:, 512 * h + off_t:512 * h + off_t + 128],
                                in_=pr[:, 512 * h + off_t:512 * h + off_t + 128],
                                channel_multiplier=-1,
                                pattern=[[1, 128]], base=0,
                                compare_op=mybir.AluOpType.is_ge,
                                fill=0.0)
                    return pr, off

                def pv(j, pr, off):
                    nc.tensor.matmul(
                        atA[0:65, off:512],
                        vw[j][:, 65 * 2 * p:65 * 2 * p + 65],
                        pr[:, off:512],
                        start=(j == 0), stop=(j == jmax))
                    nc.tensor.matmul(
                        atB[0:65, off:512],
                        vw[j][:, 65 * (2 * p + 1):65 * (2 * p + 1) + 65],
                        pr[:, 512 + off:1024],
                        start=(j == 0), stop=(j == jmax))

                pending = []
                for j in range(jmax + 1):
                    pending.append((j, *scores(j)))
                    if len(pending) > LOOKAHEAD:
                        pv(*pending.pop(0))
                for it in pending:
                    pv(*it)

                # normalize + evacuate: atn = at / denominator
                for h, at_h in ((0, atA), (1, atB)):
                    rec = nrm.tile([1, 512], F32, tag="rec")
                    nc.vector.reciprocal(rec[:], at_h[64:65, :])
                    bc = nrm.tile([64, 512], F32, tag="bc")
                    nc.gpsimd.partition_broadcast(bc[:], rec[:], channels=64)
                    nc.vector.tensor_mul(
                        atn_cur[p][64 * h:64 * (h + 1), :], at_h[0:64, :], bc[:])

            # ---- output projection for this chunk ----
            for rl in range(4):
                rt = 4 * qc + rl
                for nch in range(2):
                    po = scp.tile([128, 512], F32, tag="sc")
                    for t in range(NT):
                        nc.tensor.matmul(
                            po[:],
                            atn_cur[t][:, 128 * rl:128 * (rl + 1)],
                            wo_sb[t][:, 512 * nch:512 * (nch + 1)],
                            start=(t == 0), stop=(t == NT - 1))
                    ob = osp.tile([128, 512], BF16, tag="ob")
                    nc.vector.tensor_copy(ob[:], po[:])
                    nc.sync.dma_start(
                        out_p[128 * rt:128 * (rt + 1),
                              512 * nch:512 * (nch + 1)], ob[:])

    nc.compile()
    return nc


_NC_CACHE = {}


def get_nc():
    if "nc" not in _NC_CACHE:
        _NC_CACHE["nc"] = build_nc()
    return _NC_CACHE["nc"]


def make_in_maps(q, k, v, Wq, bq, Wk, bk, Wv, bv, Wo):
    """Host-side shard prep. Returns list of 8 per-core input dicts."""
    import ml_dtypes

    f = np.float32
    b16 = ml_dtypes.bfloat16
    q = np.asarray(q, f)
    k = np.asarray(k, f)
    v = np.asarray(v, f)
    Wq, bq = np.asarray(Wq, f), np.asarray(bq, f)
    Wk, bk = np.asarray(Wk, f), np.asarray(bk, f)
    Wv, bv = np.asarray(Wv, f), np.asarray(bv, f)
    Wo = np.asarray(Wo, f)
    qTb = [np.ascontiguousarray(q[b].T).astype(b16) for b in range(B)]
    kTb = [np.ascontiguousarray(k[b].T).astype(b16) for b in range(B)]
    vTb = [np.ascontiguousarray(v[b].T).astype(b16) for b in range(B)]
    in_maps = []
    for c in range(8):
        b, g = c // 2, c % 2
        cs = slice(DL * g, DL * (g + 1))
        in_maps.append(dict(
            qT=qTb[b],
            kT=kTb[b],
            vT=vTb[b],
            Wq_s=Wq[:, cs].astype(b16),
            Wk_s=Wk[:, cs].astype(b16),
            Wv_s=Wv[:, cs].astype(b16),
            Wo_s=Wo[cs, :].astype(b16),
            bq_s=np.ascontiguousarray(bq[cs].reshape(NT, 128).T),
            bk_s=np.ascontiguousarray(bk[cs].reshape(NT, 128).T),
            bv_bc=np.tile(bv[cs][None, :], (128, 1)),
        ))
    return in_maps


def unshard(results, bo):
    bo = np.asarray(bo, np.float32)
    out = np.empty((B, S, D), np.float32)
    for b in range(B):
        out[b] = (results[2 * b]["out_partial"].astype(np.float32)
                  + results[2 * b + 1]["out_partial"].astype(np.float32) + bo)
    return out


def kernel(q, k, v, mask, Wq, bq, Wk, bk, Wv, bv, Wo, bo, **_unused):
    nc = get_nc()
    in_maps = make_in_maps(q, k, v, Wq, bq, Wk, bk, Wv, bv, Wo)
    res = run_bass_kernel_spmd(nc, in_maps, core_ids=list(range(8))).results
    return unshard(res, bo)
